# revision 1
# baseline (speedup 1.0000x reference)
"""Trainium2 Bass kernel for nn_Attention_Encoder (conv1x1 -> time-softmax attention -> relu-GRU).

Sharding: pure data parallelism. The folded batch*ltms segment axis (64*16=1024
segments) is split across 8 NeuronCores, 128 segments per core; weights are
replicated. Each core runs the pointwise conv, per-filter softmax attention
over time, and the 256-step GRU entirely on-chip; the gates_x matmuls are fused
into the recurrence's PSUM accumulation so nothing but the x shard and the
final h ever touch DRAM.

Layouts (per core, S=128 segments):
  phase A (per segment s, bf16 matmuls, fp32 PSUM):
    x_T   [C=128p, T=256] bf16   via transpose-DMA (xbar)
    conv_T[F(2ch), T] bf16 = relu(W_c^T x_T)  (ACT evac, per-chunk bias)
    conv_N[T(2ch), F] bf16 = relu(x_T^T W_c)  (DVE evac)
    scores[F(2ch), T] = conv_N^T A ; E = exp(scores) w/ fused row-sum
    x_att [F(2ch), T] = E * (1/sum) * conv_T -> global bf16 [128, 2, T, S]
  phase B (per step t, gate-major transposed layout):
    ps_r  [r(2ch), S]       = W_r^T x_att[t] + U_r^T h    (own PSUM bank ->
                              sigmoid_r fires after 4 U-matmuls)
    ps_zh [z(2ch) rh(2ch), S], ps_xh [xh(2ch), S]
    r,z = sigmoid(...); hh = relu(xh + r*rh)  (PSUM-direct DVE reads)
    h = hh + z*(h_prev - hh)
"""

import contextlib
import os
import sys

sys.path.insert(0, "/opt/trn_rl_repo")

import numpy as np
import ml_dtypes

import concourse.bass as bass
import concourse.tile as tile
from concourse import mybir
from concourse.bass_utils import run_bass_kernel_spmd

F32 = mybir.dt.float32
F32R = mybir.dt.float32r
BF16 = mybir.dt.bfloat16
AF = mybir.ActivationFunctionType
OP = mybir.AluOpType

B, LTMS, TTS, C_IN, FF, HH = 64, 16, 256, 128, 256, 256
NCORES = 8
S = (B * LTMS) // NCORES  # 128 segments per core
T = TTS                   # 256 timesteps

# bfpack column layout (bf16): conv_w | attn_w | gru_w | gru_u | identity
BP_CW = 0
BP_AW = BP_CW + FF              # 256
BP_WG = BP_AW + 2 * T           # 768
BP_WU = BP_WG + 2 * 3 * HH      # 2304
BP_ID = BP_WU + 2 * 3 * HH      # 3840
BP_W = BP_ID + 128              # 3968


def build(zero_bias: bool) -> bass.Bass:
    nc = bass.Bass("TRN2", target_bir_lowering=False)

    x_d = nc.dram_tensor("x_shard", [S, T, C_IN], BF16, kind="ExternalInput")
    bp_d = nc.dram_tensor("bfpack", [128, BP_W], BF16, kind="ExternalInput")
    if not zero_bias:
        cb_d = nc.dram_tensor("conv_b2", [128, 2], F32, kind="ExternalInput")
        ab_d = nc.dram_tensor("attn_b", [1, T], BF16, kind="ExternalInput")
        gb_d = nc.dram_tensor("gbias", [S, 8], F32, kind="ExternalInput")
    out_d = nc.dram_tensor("h_out", [S, HH], F32, kind="ExternalOutput")

    with tile.TileContext(nc, trace_sim=bool(os.environ.get("KTRACE"))) as tc:
        with contextlib.ExitStack() as ctx:
            singles = ctx.enter_context(tc.tile_pool(name="singles", bufs=1))

            bp_sb = singles.tile([128, BP_W], BF16)
            nc.sync.dma_start(bp_sb, bp_d[:])

            cw_sb = bp_sb[:, BP_CW:BP_CW + FF]
            aw_sb = bp_sb[:, BP_AW:BP_AW + 2 * T].rearrange(
                "p (k n) -> p k n", k=2)
            wg_sb = bp_sb[:, BP_WG:BP_WG + 1536].rearrange(
                "p (k n) -> p k n", k=2)
            wu_sb = bp_sb[:, BP_WU:BP_WU + 1536].rearrange(
                "p (k n) -> p k n", k=2)
            ident_bf = bp_sb[:, BP_ID:BP_ID + 128]

            # global x_att store: [F%128, F-chunk, T, S] bf16
            xatt = singles.tile([128, 2, T, S], BF16)


            if not zero_bias:
                cb_sb = singles.tile([128, 2], F32)
                nc.sync.dma_start(cb_sb, cb_d[:])
                ab_row = singles.tile([1, T], BF16)
                nc.sync.dma_start(ab_row, ab_d[:])
                ones_col = singles.tile([1, 128], BF16)
                nc.vector.memset(ones_col, 1.0)
                gb_sb = singles.tile([128, 8], F32)
                nc.sync.dma_start(gb_sb, gb_d[:])

            # ---------------- phase A ----------------
            apool = ctx.enter_context(tc.tile_pool(name="apool", bufs=3))
            with contextlib.ExitStack() as actx:
                apsum = actx.enter_context(
                    tc.tile_pool(name="apsum", bufs=2, space="PSUM"))

                # PE warmup: consume the weight-pack DMA on PE so its queue
                # sem enters PE's vector clock (keeps matmul waits small)
                ps_w1 = apsum.tile([128, 128], BF16, tag="ps_ct", bufs=1)
                nc.tensor.transpose(ps_w1, ident_bf, ident_bf)

                for s2 in range(S // 2):
                    s = 2 * s2
                    x_t = apool.tile([128, 2, T], BF16, tag="x_t", bufs=4)
                    nc.sync.dma_start_transpose(x_t[:, 0, :], x_d[s])
                    nc.sync.dma_start_transpose(x_t[:, 1, :], x_d[s + 1])

                    # conv_T = relu(W_c^T @ x_T): [F(2ch), seg, T]
                    ps_ct = apsum.tile([128, 2, 2, T], F32, tag="ps_ct", bufs=1)
                    for m in range(2):
                        nc.tensor.matmul(
                            ps_ct[:, m, :, :], cw_sb[:, bass.ts(m, 128)],
                            x_t, start=True, stop=True)
                    conv_t = apool.tile([128, 2, 2, T], BF16, tag="conv_t")
                    # balance: chunk 0 relu on ACT, chunk 1 on DVE
                    nc.scalar.activation(
                        conv_t[:, 0, :, :], ps_ct[:, 0, :, :], AF.Relu,
                        bias=0.0 if zero_bias else cb_sb[:, 0:1])
                    if zero_bias:
                        nc.vector.tensor_scalar_max(
                            conv_t[:, 1, :, :], ps_ct[:, 1, :, :], 0.0)
                    else:
                        nc.vector.tensor_scalar(
                            conv_t[:, 1, :, :], ps_ct[:, 1, :, :],
                            cb_sb[:, 1:2], 0.0, OP.add, OP.max)

                    # conv_N = relu(x_T^T @ W_c): [seg, T-ch, F]
                    ps_cn = apsum.tile([128, 2, 2, FF], F32, tag="ps_cs", bufs=3)
                    for seg in range(2):
                        for k in range(2):
                            nc.tensor.matmul(
                                ps_cn[:, seg, k, :],
                                x_t[:, seg, bass.ts(k, 128)],
                                cw_sb, start=True, stop=True)
                    conv_n = apool.tile([128, 2, 2, FF], BF16, tag="conv_n")
                    for seg in range(2):
                        nc.vector.tensor_scalar_max(
                            conv_n[:, seg, :, :], ps_cn[:, seg, :, :], 0.0)

                    # scores = conv_N^T @ A (+ b): [seg, F-ch, T]
                    ps_s = apsum.tile([128, 2, 2, T], F32, tag="ps_cs", bufs=3)
                    for seg in range(2):
                        for m in range(2):
                            for k in range(2):
                                nc.tensor.matmul(
                                    ps_s[:, seg, m, :],
                                    conv_n[:, seg, k, bass.ts(m, 128)],
                                    aw_sb[:, k, :],
                                    start=(k == 0),
                                    stop=(k == 1) and zero_bias)
                            if not zero_bias:
                                nc.tensor.matmul(
                                    ps_s[:, seg, m, :], ones_col, ab_row,
                                    start=False, stop=True)

                    ee = apool.tile([128, 2, 2, T], BF16, tag="ee")
                    esum = apool.tile([128, 2, 2, 1], F32, tag="esum")
                    for seg in range(2):
                        for m in range(2):
                            nc.scalar.activation(
                                ee[:, seg, m, :], ps_s[:, seg, m, :], AF.Exp,
                                accum_out=esum[:, seg, m, :])
                    rinv = apool.tile([128, 2, 2, 1], F32, tag="rinv")
                    for seg in range(2):
                        for m in range(2):
                            nc.vector.reciprocal(
                                rinv[:, seg, m, :], esum[:, seg, m, :])

                    # x_att = E * rinv * conv_T -> global bf16 columns s, s+1
                    for seg in range(2):
                        for m in range(2):
                            nc.vector.scalar_tensor_tensor(
                                out=xatt[:, m, :, s + seg],
                                in0=ee[:, seg, m, :],
                                scalar=rinv[:, seg, m, :],
                                in1=conv_t[:, m, seg, :],
                                op0=OP.mult,
                                op1=OP.mult)

            # ---------------- phase B: GRU over T steps ----------------
            # gate column order in W/U: z=[0,256) m0,1 ; r=[256,512) m2,3 ;
            # h=[512,768) m4,5
            with contextlib.ExitStack() as bctx:
                hpool = bctx.enter_context(tc.tile_pool(name="hpool", bufs=2))
                gpool = bctx.enter_context(tc.tile_pool(name="gpool", bufs=3))
                bpsum = bctx.enter_context(
                    tc.tile_pool(name="bpsum", bufs=1, space="PSUM"))

                h_prev = None
                for t in range(T):
                    ps_r = bpsum.tile([128, 2, S], F32, tag="ps_r")
                    ps_z = bpsum.tile([128, 2, S], F32, tag="ps_z")
                    ps_rx = bpsum.tile([128, 4, S], F32, tag="ps_rx")

                    # x-part matmuls (independent of h)
                    for j, m in enumerate((2, 3)):      # r gates
                        for k in range(2):
                            nc.tensor.matmul(
                                ps_r[:, j, :], wg_sb[:, k, bass.ts(m, 128)],
                                xatt[:, k, t, :],
                                start=(k == 0), stop=(k == 1) and (t == 0))
                    for j, m in enumerate((0, 1)):      # z gates
                        for k in range(2):
                            nc.tensor.matmul(
                                ps_z[:, j, :], wg_sb[:, k, bass.ts(m, 128)],
                                xatt[:, k, t, :],
                                start=(k == 0), stop=(k == 1) and (t == 0))
                    for j, m in enumerate((4, 5)):      # h gate (xh)
                        for k in range(2):
                            nc.tensor.matmul(
                                ps_rx[:, 2 + j, :],
                                wg_sb[:, k, bass.ts(m, 128)],
                                xatt[:, k, t, :],
                                start=(k == 0), stop=(k == 1))
                    if t > 0:
                        # U-part: r first (gates sigmoid_r), then rh, then z
                        for j, m in enumerate((2, 3)):
                            for k in range(2):
                                nc.tensor.matmul(
                                    ps_r[:, j, :],
                                    wu_sb[:, k, bass.ts(m, 128)],
                                    h_prev[:, k, :],
                                    start=False, stop=(k == 1))
                        for j, m in enumerate((4, 5)):  # rh -> ps_rx[0:2]
                            for k in range(2):
                                nc.tensor.matmul(
                                    ps_rx[:, j, :],
                                    wu_sb[:, k, bass.ts(m, 128)],
                                    h_prev[:, k, :],
                                    start=(k == 0), stop=(k == 1))
                        for j, m in enumerate((0, 1)):
                            for k in range(2):
                                nc.tensor.matmul(
                                    ps_z[:, j, :],
                                    wu_sb[:, k, bass.ts(m, 128)],
                                    h_prev[:, k, :],
                                    start=False, stop=(k == 1))

                    xh_sb = gpool.tile([128, 2, S], BF16, tag="xh_sb")
                    if zero_bias:
                        nc.vector.tensor_copy(xh_sb, ps_rx[:, 2:4, :])
                    else:
                        for j in range(2):
                            nc.vector.tensor_scalar_add(
                                xh_sb[:, j, :], ps_rx[:, 2 + j, :],
                                gb_sb[:, 6 + j : 7 + j])

                    r = gpool.tile([128, 2, S], BF16, tag="rt")
                    z = gpool.tile([128, 2, S], BF16, tag="zt")
                    if zero_bias:
                        nc.scalar.activation(r, ps_r, AF.Sigmoid)
                        nc.scalar.activation(z, ps_z, AF.Sigmoid)
                    else:
                        for j, m in enumerate((2, 3)):
                            nc.scalar.activation(
                                r[:, j, :], ps_r[:, j, :], AF.Sigmoid,
                                bias=gb_sb[:, m : m + 1])
                        for j, m in enumerate((0, 1)):
                            nc.scalar.activation(
                                z[:, j, :], ps_z[:, j, :], AF.Sigmoid,
                                bias=gb_sb[:, m : m + 1])

                    h_new = hpool.tile([128, 2, S], BF16, tag="h")
                    hht = gpool.tile([128, 2, S], BF16, tag="hht")
                    tt = gpool.tile([128, 2, S], BF16, tag="tt")
                    if t > 0:
                        # hh = relu(xh + r*rh); rh straight from PSUM
                        if zero_bias:
                            nc.vector.tensor_mul(tt, r, ps_rx[:, 0:2, :])
                        else:
                            for j in range(2):
                                nc.vector.scalar_tensor_tensor(
                                    out=tt[:, j, :], in0=ps_rx[:, j, :],
                                    scalar=gb_sb[:, 4 + j : 5 + j],
                                    in1=r[:, j, :], op0=OP.add, op1=OP.mult)
                        nc.vector.tensor_add(tt, tt, xh_sb)
                        nc.vector.tensor_scalar_max(hht, tt, 0.0)
                        # h = hht + z*(h_prev - hht)
                        dd = gpool.tile([128, 2, S], BF16, tag="dd")
                        nc.vector.tensor_sub(dd, h_prev, hht)
                        nc.vector.tensor_mul(dd, z, dd)
                        nc.vector.tensor_add(h_new, hht, dd)
                    else:
                        if zero_bias:
                            nc.vector.tensor_scalar_max(hht, xh_sb, 0.0)
                        else:
                            for j in range(2):
                                nc.vector.tensor_scalar_mul(
                                    tt[:, j, :], r[:, j, :],
                                    gb_sb[:, 4 + j : 5 + j])
                            nc.vector.tensor_add(tt, tt, xh_sb)
                            nc.vector.tensor_scalar_max(hht, tt, 0.0)
                        wt = gpool.tile([128, 2, S], BF16, tag="tt")
                        nc.vector.tensor_scalar(wt, z, -1.0, 1.0, OP.mult,
                                                OP.add)
                        nc.vector.tensor_mul(h_new, wt, hht)

                    h_prev = h_new

                # output: transpose h back to [S, H] and store fp32
                ps_o = bpsum.tile([128, 2, S], BF16, tag="ps_r")
                for c in range(2):
                    nc.tensor.transpose(ps_o[:, c, :], h_prev[:, c, :],
                                        ident_bf)
                out_sb = gpool.tile([128, 2, 128], F32, tag="out_sb")
                nc.vector.tensor_copy(out_sb, ps_o)
                nc.sync.dma_start(
                    out_d.rearrange("s (c p) -> s c p", c=2), out_sb)

    _split_multi_waits(nc)
    return nc


def _split_multi_waits(nc: bass.Bass):
    """This walrus encodes at most ONE semaphore wait per ISA instruction.
    Tile's sem assignment can attach several; hoist the excess onto
    preceding same-engine NoOp carriers (the sequencer executes them in
    order, so semantics are identical)."""
    fn = nc.m.functions[0]
    for blk in fn.blocks:
        insts = list(blk.instructions)
        out = []
        changed = False
        for inst in insts:
            si = inst.sync_info
            waits = list(si.on_wait) if si is not None else []
            if len(waits) > 1:
                changed = True
                for w in waits[:-1]:
                    out.append(mybir.InstNoOp(
                        name=f"I-wsplit-{nc.next_id()}",
                        engine=inst.engine,
                        ins=[], outs=[],
                        sync_info=mybir.SyncInfo(on_wait=[w], on_update=[]),
                    ))
                inst.sync_info = mybir.SyncInfo(
                    on_wait=[waits[-1]], on_update=list(si.on_update))
            out.append(inst)
        if changed:
            blk.instructions = out


_CACHE = {}


def _get_nc(zero_bias: bool) -> bass.Bass:
    key = zero_bias
    if key not in _CACHE:
        _CACHE[key] = build(zero_bias)
    return _CACHE[key]


def _pack_weights(conv_w, attn_w, gru_w, gru_u):
    bf = ml_dtypes.bfloat16
    cw = (conv_w[0] if conv_w.ndim == 3 else conv_w).astype(bf)  # [128, 256]
    aw = attn_w.astype(bf).reshape(2, 128, T).transpose(1, 0, 2).reshape(
        128, 2 * T)
    wg = gru_w.astype(bf).reshape(2, 128, 768).transpose(1, 0, 2).reshape(
        128, 1536)
    wu = gru_u.astype(bf).reshape(2, 128, 768).transpose(1, 0, 2).reshape(
        128, 1536)
    ident = np.eye(128, dtype=np.float32).astype(bf)
    return np.ascontiguousarray(
        np.concatenate([cw, aw, wg, wu, ident], axis=1), bf)


def kernel(x, conv_w, conv_b, attn_w, attn_b, gru_w, gru_u, gru_b):
    x = np.asarray(x, dtype=np.float32)
    conv_w = np.asarray(conv_w, dtype=np.float32)
    conv_b = np.asarray(conv_b, dtype=np.float32)
    attn_w = np.asarray(attn_w, dtype=np.float32)
    attn_b = np.asarray(attn_b, dtype=np.float32)
    gru_w = np.asarray(gru_w, dtype=np.float32)
    gru_u = np.asarray(gru_u, dtype=np.float32)
    gru_b = np.asarray(gru_b, dtype=np.float32)

    zero_bias = (
        not conv_b.any() and not attn_b.any() and not gru_b.any())

    nc = _get_nc(zero_bias)

    xs_bf = x.reshape(B * LTMS, T, C_IN).astype(ml_dtypes.bfloat16)
    bfpack = _pack_weights(conv_w, attn_w, gru_w, gru_u)

    in_maps = []
    for c in range(NCORES):
        m = {
            "x_shard": np.ascontiguousarray(xs_bf[c * S : (c + 1) * S]),
            "bfpack": bfpack,
        }
        if not zero_bias:
            bi, br = gru_b[0], gru_b[1]
            comb = bi + br
            gb = np.zeros((128, 8), np.float32)
            for ch in range(4):
                gb[:, ch] = comb[ch * 128 : (ch + 1) * 128]
            gb[:, 4] = br[512:640]
            gb[:, 5] = br[640:768]
            gb[:, 6] = bi[512:640]
            gb[:, 7] = bi[640:768]
            m["conv_b2"] = np.ascontiguousarray(
                conv_b.reshape(2, 128).T, np.float32)
            m["attn_b"] = attn_b.reshape(1, T).astype(ml_dtypes.bfloat16)
            m["gbias"] = gb
        in_maps.append(m)

    res = run_bass_kernel_spmd(nc, in_maps, core_ids=list(range(NCORES)))
    outs = [res.results[c]["h_out"] for c in range(NCORES)]
    h = np.concatenate(outs, axis=0)  # [1024, 256]
    return h.reshape(B, LTMS, HH).astype(np.float32)


if __name__ == "__main__":
    nc = _get_nc(True)
    print("built ok")



# revision 30
# speedup vs baseline: 1.1928x; 1.1928x over previous
"""Trainium2 Bass kernel for nn_Attention_Encoder (conv1x1 -> time-softmax attention -> relu-GRU).

Sharding: pure data parallelism. The folded batch*ltms segment axis (64*16=1024
segments) is split across 8 NeuronCores, 128 segments per core; weights are
replicated. Each core runs the pointwise conv, per-filter softmax attention
over time, and the 256-step GRU entirely on-chip; the gates_x matmuls are fused
into the recurrence's PSUM accumulation so nothing but the x shard and the
final h ever touch DRAM.

Layouts (per core, S=128 segments):
  phase A (per segment pair, bf16 matmuls, fp32 PSUM):
    x_T   [C=128p, T=256] bf16   via transpose-DMA (xbar)
    conv_T[F(2ch), seg, T] bf16 = relu(W_c^T x_T)  (evac split DVE/Pool)
    conv_N[seg, T-ch, F] bf16 = relu(x_T^T W_c)    (evac split DVE/Pool)
    scores[seg, F-ch, T] = conv_N^T A ; E = exp(scores) w/ fused row-sum (ACT)
    x_att stored [128, F-ch, S, T] bf16 (T packed -> 4x DVE STT writes);
    the rinv+apply for pair i is issued in iteration i+1 so the DVE queue
    never stalls waiting on ACT's exp.
  phase B (per step t, gate-major transposed layout, double-buffered PSUM):
    ps_r/ps_z [gate(2ch), S], ps_rx [rh(2ch) xh(2ch), S]
    x-part matmuls for t+1 issued right after U-part of t (other PSUM buf),
    so they overlap the DVE chain; Pool evacuates xh(t+1) to SBUF bf16.
    chain: sig_r (ACT) -> u=r*rh(PSUM) -> +xh -> relu -> d=h-hh -> z*d -> h
    with all SBUF operands bf16 (2x/4x DVE modes).
"""

import contextlib
import os
import sys

sys.path.insert(0, "/opt/trn_rl_repo")

import numpy as np
import ml_dtypes

import concourse.bass as bass
import concourse.tile as tile
from concourse import mybir
from concourse.bass_utils import run_bass_kernel_spmd

F32 = mybir.dt.float32
F32R = mybir.dt.float32r
BF16 = mybir.dt.bfloat16
AF = mybir.ActivationFunctionType
OP = mybir.AluOpType

def _flat(ap):
    return ap.rearrange("p a b -> p (a b)")


B, LTMS, TTS, C_IN, FF, HH = 64, 16, 256, 128, 256, 256
NCORES = 8
S = (B * LTMS) // NCORES  # 128 segments per core
T = TTS                   # 256 timesteps

# bfpack column layout (bf16): conv_w | attn_w | gru_w | gru_u | identity
BP_CW = 0
BP_AW = BP_CW + FF              # 256
BP_WG = BP_AW + 2 * T           # 768
BP_WU = BP_WG + 2 * 3 * HH      # 2304
BP_ID = BP_WU + 2 * 3 * HH      # 3840
BP_W = BP_ID + 128              # 3968


def build(zero_bias: bool) -> bass.Bass:
    nc = bass.Bass("TRN2", target_bir_lowering=False)

    x_d = nc.dram_tensor("x_shard", [S, T, C_IN], BF16, kind="ExternalInput")
    bp_d = nc.dram_tensor("bfpack", [128, BP_W], BF16, kind="ExternalInput")
    if not zero_bias:
        cb_d = nc.dram_tensor("conv_b2", [128, 2], F32, kind="ExternalInput")
        ab_d = nc.dram_tensor("attn_b", [1, T], BF16, kind="ExternalInput")
        gb_d = nc.dram_tensor("gbias", [S, 8], F32, kind="ExternalInput")
    out_d = nc.dram_tensor("h_out", [S, HH], F32, kind="ExternalOutput")

    with tile.TileContext(nc, trace_sim=bool(os.environ.get("KTRACE"))) as tc:
        with contextlib.ExitStack() as ctx:
            singles = ctx.enter_context(tc.tile_pool(name="singles", bufs=1))

            bp_sb = singles.tile([128, BP_W], BF16)
            nc.sync.dma_start(bp_sb, bp_d[:])

            cw_sb = bp_sb[:, BP_CW:BP_CW + FF]
            aw_sb = bp_sb[:, BP_AW:BP_AW + 2 * T].rearrange(
                "p (k n) -> p k n", k=2)
            wg_sb = bp_sb[:, BP_WG:BP_WG + 1536].rearrange(
                "p (k n) -> p k n", k=2)
            wu_sb = bp_sb[:, BP_WU:BP_WU + 1536].rearrange(
                "p (k n) -> p k n", k=2)
            ident_bf = bp_sb[:, BP_ID:BP_ID + 128]

            # global x_att store: [F%128, F-chunk, S, T] bf16 (T packed)
            xatt = singles.tile([128, 2, S, T], BF16)

            # per-partition scalar constants for the fused custom-DVE ops
            zero_col = singles.tile([128, 1], F32)
            nc.vector.memset(zero_col, 0.0)
            one_col2 = singles.tile([128, 1], F32)
            nc.vector.memset(one_col2, 1.0)

            if not zero_bias:
                cb_sb = singles.tile([128, 2], F32)
                nc.sync.dma_start(cb_sb, cb_d[:])
                ab_row = singles.tile([1, T], BF16)
                nc.sync.dma_start(ab_row, ab_d[:])
                ones_col = singles.tile([1, 128], BF16)
                nc.vector.memset(ones_col, 1.0)
                gb_sb = singles.tile([128, 8], F32)
                nc.sync.dma_start(gb_sb, gb_d[:])

            # ---------------- phase A ----------------
            apool = ctx.enter_context(tc.tile_pool(name="apool", bufs=3))
            with contextlib.ExitStack() as actx:
                apsum = actx.enter_context(
                    tc.tile_pool(name="apsum", bufs=2, space="PSUM"))

                # PE warmup: consume the weight-pack DMA on PE so its queue
                # sem enters PE's vector clock (keeps matmul waits small)
                ps_w1 = apsum.tile([128, 128], BF16, tag="ps_ct", bufs=1)
                nc.tensor.transpose(ps_w1, ident_bf, ident_bf)

                # deferred normalize+apply state of the previous pair
                prev = None

                def apply_xatt(st):
                    # normalize+apply on Pool (all-SBUF; Pool has no STT, so
                    # two ops: en = ee*rinv, then xatt = en*conv_t)
                    ee_p, esum_p, rinv_p, src_p, s_p = st
                    nc.vector.reciprocal(
                        rinv_p.rearrange("p a b c -> p (a b c)"),
                        esum_p.rearrange("p a b c -> p (a b c)"))
                    en = apool.tile([128, 2, 2, T], BF16, tag="eenorm")
                    for seg in range(2):
                        for m in range(2):
                            nc.gpsimd.tensor_scalar_mul(
                                en[:, seg, m, :], ee_p[:, seg, m, :],
                                rinv_p[:, seg, m, :])
                            nc.gpsimd.tensor_mul(
                                xatt[:, m, s_p + seg, :],
                                en[:, seg, m, :],
                                src_p[:, m, seg, :])

                for s2 in range(S // 2):
                    s = 2 * s2
                    x_t = apool.tile([128, 2, T], BF16, tag="x_t", bufs=4)
                    nc.sync.dma_start_transpose(x_t[:, 0, :], x_d[s])
                    nc.sync.dma_start_transpose(x_t[:, 1, :], x_d[s + 1])

                    # conv_T (pre-relu) = W_c^T @ x_T: [F(2ch), seg, T]
                    ps_ct = apsum.tile([128, 2, 2, T], F32, tag="ps_ct", bufs=1)
                    for m in range(2):
                        nc.tensor.matmul(
                            ps_ct[:, m, :, :], cw_sb[:, bass.ts(m, 128)],
                            x_t, start=True, stop=True)
                    # GPSIMD cannot touch PSUM on TRN2: all evacs on DVE
                    conv_t = apool.tile([128, 2, 2, T], BF16, tag="conv_t")
                    if zero_bias:
                        nc.vector.tensor_scalar_max(
                            conv_t[:, 0, :, :], ps_ct[:, 0, :, :], 0.0)
                        nc.vector.tensor_scalar_max(
                            conv_t[:, 1, :, :], ps_ct[:, 1, :, :], 0.0)
                    else:
                        nc.vector.tensor_scalar(
                            conv_t[:, 0, :, :], ps_ct[:, 0, :, :],
                            cb_sb[:, 0:1], 0.0, OP.add, OP.max)
                        nc.vector.tensor_scalar(
                            conv_t[:, 1, :, :], ps_ct[:, 1, :, :],
                            cb_sb[:, 1:2], 0.0, OP.add, OP.max)
                    ct_src = conv_t

                    # conv_N = relu(x_T^T @ W_c): [seg, T-ch, F]
                    ps_cn = apsum.tile([128, 2, 2, FF], F32, tag="ps_cn", bufs=1)
                    for seg in range(2):
                        for k in range(2):
                            nc.tensor.matmul(
                                ps_cn[:, seg, k, :],
                                x_t[:, seg, bass.ts(k, 128)],
                                cw_sb, start=True, stop=True)
                    conv_n = apool.tile([128, 2, 2, FF], BF16, tag="conv_n")
                    nc.vector.tensor_scalar_max(
                        conv_n[:, 0, :, :], ps_cn[:, 0, :, :], 0.0)
                    nc.vector.tensor_scalar_max(
                        conv_n[:, 1, :, :], ps_cn[:, 1, :, :], 0.0)

                    # scores = conv_N^T @ A (+ b): [seg, F-ch, T]
                    ps_s = apsum.tile([128, 2, 2, T], F32, tag="ps_s", bufs=2)
                    for seg in range(2):
                        for m in range(2):
                            for k in range(2):
                                nc.tensor.matmul(
                                    ps_s[:, seg, m, :],
                                    conv_n[:, seg, k, bass.ts(m, 128)],
                                    aw_sb[:, k, :],
                                    start=(k == 0),
                                    stop=(k == 1) and zero_bias)
                            if not zero_bias:
                                nc.tensor.matmul(
                                    ps_s[:, seg, m, :], ones_col, ab_row,
                                    start=False, stop=True)

                    ee = apool.tile([128, 2, 2, T], BF16, tag="ee")
                    esum = apool.tile([128, 2, 2, 1], F32, tag="esum")
                    for seg in range(2):
                        for m in range(2):
                            nc.scalar.activation(
                                ee[:, seg, m, :], ps_s[:, seg, m, :], AF.Exp,
                                accum_out=esum[:, seg, m, :])

                    # normalize+apply for the PREVIOUS pair (keeps DVE's
                    # in-order queue from stalling on this pair's exps)
                    if prev is not None:
                        apply_xatt(prev)
                    rinv = apool.tile([128, 2, 2, 1], F32, tag="rinv")
                    prev = (ee, esum, rinv, ct_src, s)

                apply_xatt(prev)

            # ---------------- phase B: GRU over T steps ----------------
            # gate column order in W/U: z=[0,256) m0,1 ; r=[256,512) m2,3 ;
            # h=[512,768) m4,5
            with contextlib.ExitStack() as bctx:
                hpool = bctx.enter_context(tc.tile_pool(name="hpool", bufs=2))
                gpool = bctx.enter_context(tc.tile_pool(name="gpool", bufs=2))
                bpsum = bctx.enter_context(
                    tc.tile_pool(name="bpsum", bufs=2, space="PSUM"))

                def alloc_ps():
                    return (bpsum.tile([128, 2, S], F32, tag="ps_r",
                                       name="ps_r"),
                            bpsum.tile([128, 2, S], F32, tag="ps_z",
                                       name="ps_z"),
                            bpsum.tile([128, 4, S], F32, tag="ps_rx",
                                       name="ps_rx"))

                def x_mms(ps_r, ps_z, ps_rx, t, with_stop):
                    for j, m in enumerate((2, 3)):      # r gates
                        for k in range(2):
                            nc.tensor.matmul(
                                ps_r[:, j, :], wg_sb[:, k, bass.ts(m, 128)],
                                xatt[:, k, :, t],
                                start=(k == 0), stop=(k == 1) and with_stop)
                    for j, m in enumerate((0, 1)):      # z gates
                        for k in range(2):
                            nc.tensor.matmul(
                                ps_z[:, j, :], wg_sb[:, k, bass.ts(m, 128)],
                                xatt[:, k, :, t],
                                start=(k == 0), stop=(k == 1) and with_stop)
                    for j, m in enumerate((4, 5)):      # h gate (xh)
                        for k in range(2):
                            nc.tensor.matmul(
                                ps_rx[:, 2 + j, :],
                                wg_sb[:, k, bass.ts(m, 128)],
                                xatt[:, k, :, t],
                                start=(k == 0), stop=(k == 1))

                def xh_evac(ps_rx, xh_sb):
                    # ACT evacuates xh (x-part of h gate) to SBUF bf16: ACT
                    # only has the two sigmoids per step, and DVE ordering
                    # would let the scheduler wedge this inside the chain
                    if zero_bias:
                        nc.scalar.activation(xh_sb, ps_rx[:, 2:4, :], AF.Copy)
                    else:
                        for j in range(2):
                            nc.scalar.activation(
                                xh_sb[:, j, :], ps_rx[:, 2 + j, :],
                                AF.Identity, bias=gb_sb[:, 6 + j: 7 + j])

                # prologue: t=0 x-part matmuls (full groups for r/z: no U)
                ps_r, ps_z, ps_rx = alloc_ps()
                x_mms(ps_r, ps_z, ps_rx, 0, with_stop=True)
                xh_sb = gpool.tile([128, 2, S], BF16, tag="xh_sb")
                xh_evac(ps_rx, xh_sb)

                h_prev = None
                for t in range(T):
                    if t > 0:
                        # U-part order: r (gates sigmoid_r), z (gates
                        # sigmoid_z early), then rh
                        for j, m in enumerate((2, 3)):
                            for k in range(2):
                                nc.tensor.matmul(
                                    ps_r[:, j, :],
                                    wu_sb[:, k, bass.ts(m, 128)],
                                    h_prev[:, k, :],
                                    start=False, stop=(k == 1))
                        for j, m in enumerate((0, 1)):
                            for k in range(2):
                                nc.tensor.matmul(
                                    ps_z[:, j, :],
                                    wu_sb[:, k, bass.ts(m, 128)],
                                    h_prev[:, k, :],
                                    start=False, stop=(k == 1))
                        for j, m in enumerate((4, 5)):  # rh -> ps_rx[0:2]
                            for k in range(2):
                                nc.tensor.matmul(
                                    ps_rx[:, j, :],
                                    wu_sb[:, k, bass.ts(m, 128)],
                                    h_prev[:, k, :],
                                    start=(k == 0), stop=(k == 1))

                    w = gpool.tile([128, 2, S], BF16, tag="wt")
                    need_r = (t > 0) or not zero_bias
                    if need_r:
                        r = gpool.tile([128, 2, S], BF16, tag="rt")
                        if zero_bias:
                            nc.scalar.activation(r, ps_r, AF.Sigmoid)
                        else:
                            for j, m in enumerate((2, 3)):
                                nc.scalar.activation(
                                    r[:, j, :], ps_r[:, j, :], AF.Sigmoid,
                                    bias=gb_sb[:, m: m + 1])
                    # z-gate weights/bias are packed NEGATED, so this sigmoid
                    # yields w = sigmoid(-(xz+rz)) = 1 - z directly
                    if zero_bias:
                        nc.scalar.activation(w, ps_z, AF.Sigmoid)
                    else:
                        for j, m in enumerate((0, 1)):
                            nc.scalar.activation(
                                w[:, j, :], ps_z[:, j, :], AF.Sigmoid,
                                bias=gb_sb[:, m: m + 1])

                    # p = z*h_prev = h_prev - w*h_prev on Pool (off the chain)
                    if t > 0:
                        q = gpool.tile([128, 2, S], BF16, tag="qt")
                        p = gpool.tile([128, 2, S], BF16, tag="pt")
                        nc.gpsimd.tensor_mul(q, w, h_prev)
                        nc.gpsimd.tensor_sub(p, h_prev, q)

                    # next step's x-part into the other PSUM buffers; runs on
                    # PE while this step's DVE chain executes
                    if t + 1 < T:
                        ps_r2, ps_z2, ps_rx2 = alloc_ps()
                        x_mms(ps_r2, ps_z2, ps_rx2, t + 1, with_stop=False)

                    # ---- DVE chain: u -> tt -> v=w*relu(tt) -> h=v+p ----
                    h_new = hpool.tile([128, 2, S], BF16, tag="h")
                    if t > 0:
                        u = gpool.tile([128, 2, S], BF16, tag="ut")
                        tt = gpool.tile([128, 2, S], BF16, tag="tt")
                        # u = r * (rh [+ br_h]); rh read straight from PSUM
                        if zero_bias:
                            nc.vector.tensor_mul(u, r, ps_rx[:, 0:2, :])
                        else:
                            for j in range(2):
                                nc.vector.scalar_tensor_tensor(
                                    out=u[:, j, :], in0=ps_rx[:, j, :],
                                    scalar=gb_sb[:, 4 + j: 5 + j],
                                    in1=r[:, j, :], op0=OP.add, op1=OP.mult)
                        nc.vector.tensor_add(tt, u, xh_sb)
                        hht = gpool.tile([128, 2, S], BF16, tag="hht")
                        nc.vector.tensor_scalar_max(hht, tt, 0.0)
                        v = gpool.tile([128, 2, S], BF16, tag="vt")
                        nc.vector.tensor_mul(v, w, hht)
                        nc.vector.tensor_add(h_new, v, p)
                    else:
                        hht = gpool.tile([128, 2, S], BF16, tag="hht")
                        if zero_bias:
                            # h0 = (1-z) * relu(xh)
                            nc.vector.tensor_scalar_max(hht, xh_sb, 0.0)
                        else:
                            tt = gpool.tile([128, 2, S], BF16, tag="tt")
                            for j in range(2):
                                nc.vector.tensor_scalar_mul(
                                    tt[:, j, :], r[:, j, :],
                                    gb_sb[:, 4 + j: 5 + j])
                            nc.vector.tensor_add(tt, tt, xh_sb)
                            nc.vector.tensor_scalar_max(hht, tt, 0.0)
                        nc.vector.tensor_mul(h_new, w, hht)

                    h_prev = h_new
                    if t + 1 < T:
                        # xh(t+1) PSUM->SBUF evac after the chain ops: lands
                        # in DVE's idle window while PE/ACT run step t+1 head
                        xh_sb2 = gpool.tile([128, 2, S], BF16, tag="xh_sb")
                        xh_evac(ps_rx2, xh_sb2)
                        ps_r, ps_z, ps_rx = ps_r2, ps_z2, ps_rx2
                        xh_sb = xh_sb2

                # output: transpose h back to [S, H] and store fp32
                ps_o = bpsum.tile([128, 2, S], BF16, tag="ps_r")
                for c in range(2):
                    nc.tensor.transpose(ps_o[:, c, :], h_prev[:, c, :],
                                        ident_bf)
                out_sb = gpool.tile([128, 2, 128], F32, tag="out_sb")
                nc.vector.tensor_copy(out_sb, ps_o)
                nc.sync.dma_start(
                    out_d.rearrange("s (c p) -> s c p", c=2), out_sb)

    _split_multi_waits(nc)
    return nc


def _split_multi_waits(nc: bass.Bass):
    """This walrus encodes at most ONE semaphore wait per ISA instruction.
    Tile's sem assignment can attach several; hoist the excess onto
    preceding same-engine NoOp carriers (the sequencer executes them in
    order, so semantics are identical)."""
    fn = nc.m.functions[0]
    for blk in fn.blocks:
        insts = list(blk.instructions)
        out = []
        changed = False
        for inst in insts:
            si = inst.sync_info
            waits = list(si.on_wait) if si is not None else []
            if len(waits) > 1:
                changed = True
                for w in waits[:-1]:
                    out.append(mybir.InstNoOp(
                        name=f"I-wsplit-{nc.next_id()}",
                        engine=inst.engine,
                        ins=[], outs=[],
                        sync_info=mybir.SyncInfo(on_wait=[w], on_update=[]),
                    ))
                inst.sync_info = mybir.SyncInfo(
                    on_wait=[waits[-1]], on_update=list(si.on_update))
            out.append(inst)
        if changed:
            blk.instructions = out


_CACHE = {}


def _get_nc(zero_bias: bool) -> bass.Bass:
    key = zero_bias
    if key not in _CACHE:
        _CACHE[key] = build(zero_bias)
    return _CACHE[key]


def _pack_weights(conv_w, attn_w, gru_w, gru_u):
    bf = ml_dtypes.bfloat16
    cw = (conv_w[0] if conv_w.ndim == 3 else conv_w).astype(bf)  # [128, 256]
    aw = attn_w.astype(bf).reshape(2, 128, T).transpose(1, 0, 2).reshape(
        128, 2 * T)
    # z-gate columns [0, 256) negated: sigmoid(ps_z) then yields 1-z
    gw = gru_w.copy()
    gw[:, 0:256] = -gw[:, 0:256]
    gu = gru_u.copy()
    gu[:, 0:256] = -gu[:, 0:256]
    wg = gw.astype(bf).reshape(2, 128, 768).transpose(1, 0, 2).reshape(
        128, 1536)
    wu = gu.astype(bf).reshape(2, 128, 768).transpose(1, 0, 2).reshape(
        128, 1536)
    ident = np.eye(128, dtype=np.float32).astype(bf)
    return np.ascontiguousarray(
        np.concatenate([cw, aw, wg, wu, ident], axis=1), bf)


def kernel(x, conv_w, conv_b, attn_w, attn_b, gru_w, gru_u, gru_b):
    x = np.asarray(x, dtype=np.float32)
    conv_w = np.asarray(conv_w, dtype=np.float32)
    conv_b = np.asarray(conv_b, dtype=np.float32)
    attn_w = np.asarray(attn_w, dtype=np.float32)
    attn_b = np.asarray(attn_b, dtype=np.float32)
    gru_w = np.asarray(gru_w, dtype=np.float32)
    gru_u = np.asarray(gru_u, dtype=np.float32)
    gru_b = np.asarray(gru_b, dtype=np.float32)

    zero_bias = (
        not conv_b.any() and not attn_b.any() and not gru_b.any())

    nc = _get_nc(zero_bias)

    xs_bf = x.reshape(B * LTMS, T, C_IN).astype(ml_dtypes.bfloat16)
    bfpack = _pack_weights(conv_w, attn_w, gru_w, gru_u)

    in_maps = []
    for c in range(NCORES):
        m = {
            "x_shard": np.ascontiguousarray(xs_bf[c * S: (c + 1) * S]),
            "bfpack": bfpack,
        }
        if not zero_bias:
            bi, br = gru_b[0], gru_b[1]
            comb = bi + br
            gb = np.zeros((128, 8), np.float32)
            for ch in range(4):
                gb[:, ch] = comb[ch * 128: (ch + 1) * 128]
            gb[:, 0:2] = -gb[:, 0:2]  # negated z bias (w-sigmoid)
            gb[:, 4] = br[512:640]
            gb[:, 5] = br[640:768]
            gb[:, 6] = bi[512:640]
            gb[:, 7] = bi[640:768]
            m["conv_b2"] = np.ascontiguousarray(
                conv_b.reshape(2, 128).T, np.float32)
            m["attn_b"] = attn_b.reshape(1, T).astype(ml_dtypes.bfloat16)
            m["gbias"] = gb
        in_maps.append(m)

    res = run_bass_kernel_spmd(nc, in_maps, core_ids=list(range(NCORES)))
    outs = [res.results[c]["h_out"] for c in range(NCORES)]
    h = np.concatenate(outs, axis=0)  # [1024, 256]
    return h.reshape(B, LTMS, HH).astype(np.float32)


if __name__ == "__main__":
    nc = _get_nc(True)
    print("built ok")
